# revision 5
# baseline (speedup 1.0000x reference)
"""AdMSoftmaxLoss fused distributed kernel for 8 TRN2 NeuronCores.

Math (reference):
    xn = x / ||x||                     # row-L2-normalized embeddings
    wf = xn @ W.T                      # [N, C] logits
    tgt = wf[i, y_i]
    num = S * (tgt - M)
    excl = sum_c exp(S*wf) - exp(S*tgt)
    L = num - log(exp(num) + excl);  loss = -mean(L)

Strategy: pure data-parallel over N (16384 rows -> 2048/core), no
collectives.  Each core computes the full [2048, 10000] logit block in
PSUM tiles (bf16 matmul, fp32 accumulate) and fuses exp + row-sum into
single ScalarEngine activation instructions:
    exp(S*r_i*raw_logit) via per-partition scale (S*r_i), accum_out=row-sum
The normalization 1/||x_i|| is folded into the activation scale, so x is
never normalized explicitly.  The target logit is computed exactly from a
host-side gather G = W[labels] via a DVE dot-product (x.G per row).
Per-row L values are DMA'd out; the host just concatenates and means.
"""

import numpy as np
import ml_dtypes

import concourse.mybir as mybir
import concourse.tile as tile
from concourse import bacc
from concourse.bass_utils import run_bass_kernel_spmd

N, D, C = 16384, 256, 10000
S, M = 30.0, 0.4
NCORES = 8
NS = N // NCORES      # 2048 rows per core
NT = NS // 128        # 16 n-tiles of 128 rows
KT = D // 128         # 2 k-slices
CG = [2048, 2048, 2048, 2048, 1808]   # class-dim groups (sum = C)
assert sum(CG) == C

_F32 = mybir.dt.float32
_BF16 = mybir.dt.bfloat16


def _build_nc(ns=NS, cg=tuple(CG), c=C):
    nt = ns // 128
    cg = list(cg)
    assert sum(cg) == c
    nc = bacc.Bacc("TRN2", target_bir_lowering=False)
    mult = mybir.AluOpType.mult
    add = mybir.AluOpType.add
    AF = mybir.ActivationFunctionType
    NT, C = nt, c  # noqa: N806 (shadow module constants for the body below)
    CG, NS = cg, ns  # noqa: N806

    xt_ext = nc.declare_dram_parameter("xt", [KT, 128, NS], _BF16, isOutput=False)
    wt_ext = nc.declare_dram_parameter("wt", [KT, 128, C], _BF16, isOutput=False)
    xf_ext = nc.declare_dram_parameter("xf", [NT, 128, D], _F32, isOutput=False)
    g_ext = nc.declare_dram_parameter("g", [NT, 128, D], _F32, isOutput=False)
    out_ext = nc.declare_dram_parameter("out", [128, NT], _F32, isOutput=True)

    with tile.TileContext(nc) as tc:
        with (
            tc.tile_pool(name="big", bufs=1) as big,
            tc.tile_pool(name="stat", bufs=1) as stat,
            tc.tile_pool(name="scr", bufs=1) as scr,
            tc.tile_pool(name="psum", bufs=2, space="PSUM") as psum,
        ):
            # ---- resident input tiles ----
            wt_sb = big.tile([128, KT, C], _BF16)
            xt_sb = big.tile([128, KT, NS], _BF16)
            xf_sb = big.tile([128, NT, D], _F32)
            g_sb = big.tile([128, NT, D], _F32)
            for k in range(KT):
                c0 = 0
                for w in CG:
                    nc.sync.dma_start(
                        out=wt_sb[:, k, c0 : c0 + w], in_=wt_ext[k, :, c0 : c0 + w]
                    )
                    c0 += w
                nc.sync.dma_start(out=xt_sb[:, k, :], in_=xt_ext[k])
            for t in range(NT):
                nc.sync.dma_start(out=xf_sb[:, t, :], in_=xf_ext[t])
                nc.sync.dma_start(out=g_sb[:, t, :], in_=g_ext[t])

            # ---- phase 1: row stats (per-partition scalars, [128, NT]) ----
            ss = stat.tile([128, NT], _F32)     # sum of squares
            rawt = stat.tile([128, NT], _F32)   # x . W[y]  (unnormalized target)
            dot_scr = scr.tile([128, D], _F32)
            for t in range(NT):
                nc.vector.tensor_mul(dot_scr, xf_sb[:, t, :], xf_sb[:, t, :])
                nc.vector.reduce_sum(
                    ss[:, t : t + 1], dot_scr, axis=mybir.AxisListType.X
                )
                nc.vector.tensor_mul(dot_scr, xf_sb[:, t, :], g_sb[:, t, :])
                nc.vector.reduce_sum(
                    rawt[:, t : t + 1], dot_scr, axis=mybir.AxisListType.X
                )
            inv = stat.tile([128, NT], _F32)
            nc.vector.reciprocal(inv, ss)
            r = stat.tile([128, NT], _F32)
            nc.scalar.activation(r, inv, AF.Sqrt)          # r = 1/||x||
            sr = stat.tile([128, NT], _F32)
            nc.vector.tensor_scalar_mul(sr, r, S)          # S * r
            st = stat.tile([128, NT], _F32)
            nc.vector.tensor_mul(st, sr, rawt)             # S * tgt
            num = stat.tile([128, NT], _F32)
            nc.vector.tensor_scalar_add(num, st, -S * M)   # S * (tgt - M)
            expn = stat.tile([128, NT], _F32)
            nc.scalar.activation(expn, num, AF.Exp)
            expt = stat.tile([128, NT], _F32)
            nc.scalar.activation(expt, st, AF.Exp)

            # ---- phase 2: logits + fused exp/row-sum ----
            esum = stat.tile([128, NT * len(CG)], _F32)
            exp_scr = scr.tile([128, 2048], _F32)
            for gi, w in enumerate(CG):
                c0 = sum(CG[:gi])
                for t in range(NT):
                    pt = psum.tile([128, 2048], _F32, tag="pt")
                    for b0 in range(0, w, 512):
                        bw = min(512, w - b0)
                        for k in range(KT):
                            nc.tensor.matmul(
                                pt[:, b0 : b0 + bw],
                                xt_sb[:, k, t * 128 : (t + 1) * 128],
                                wt_sb[:, k, c0 + b0 : c0 + b0 + bw],
                                start=(k == 0),
                                stop=(k == KT - 1),
                            )
                    idx = t * len(CG) + gi
                    nc.scalar.activation(
                        exp_scr[:, :w],
                        pt[:, :w],
                        AF.Exp,
                        scale=sr[:, t : t + 1],
                        accum_out=esum[:, idx : idx + 1],
                    )

            # ---- phase 3: combine, log, write out ----
            esum_v = esum.rearrange("p (t g) -> p t g", g=len(CG))
            et = stat.tile([128, NT], _F32)
            for t in range(NT):
                nc.vector.reduce_sum(
                    et[:, t : t + 1], esum_v[:, t, :], axis=mybir.AxisListType.X
                )
            denom = stat.tile([128, NT], _F32)
            nc.vector.tensor_add(denom, et, expn)
            nc.vector.tensor_sub(denom, denom, expt)
            lg = stat.tile([128, NT], _F32)
            nc.scalar.activation(lg, denom, AF.Ln)
            L = stat.tile([128, NT], _F32)
            nc.vector.tensor_sub(L, num, lg)
            nc.sync.dma_start(out=out_ext[:], in_=L)

    nc.finalize()
    return nc


_NC_CACHE = None


def _get_nc():
    global _NC_CACHE
    if _NC_CACHE is None:
        _NC_CACHE = _build_nc()
    return _NC_CACHE


def make_in_maps(x, labels, W):
    x = np.asarray(x, dtype=np.float32)
    W = np.asarray(W, dtype=np.float32)
    labels = np.asarray(labels)
    wt = np.ascontiguousarray(W.T).astype(ml_dtypes.bfloat16).reshape(KT, 128, C)
    in_maps = []
    for i in range(NCORES):
        xs = x[i * NS : (i + 1) * NS]                       # [NS, D]
        ls = labels[i * NS : (i + 1) * NS]
        xt = np.ascontiguousarray(xs.T).astype(ml_dtypes.bfloat16).reshape(KT, 128, NS)
        xf = np.ascontiguousarray(xs).reshape(NT, 128, D)
        g = np.ascontiguousarray(W[ls]).reshape(NT, 128, D)
        in_maps.append({"xt": xt, "wt": wt, "xf": xf, "g": g})
    return in_maps


def run_device(x, labels, W, **kwargs):
    nc = _get_nc()
    in_maps = make_in_maps(x, labels, W)
    res = run_bass_kernel_spmd(nc, in_maps, list(range(NCORES)), **kwargs)
    return res


def finish(res):
    parts = []
    for i in range(NCORES):
        o = res.results[i]["out"]            # [128, NT]; row = t*128 + p
        parts.append(np.asarray(o).T.reshape(-1))
    L = np.concatenate(parts)
    return np.asarray(-np.mean(L), dtype=np.float32)


def kernel(x, labels, W):
    return finish(run_device(x, labels, W))


# revision 6
# speedup vs baseline: 1.0912x; 1.0912x over previous
"""AdMSoftmaxLoss fused distributed kernel for 8 TRN2 NeuronCores.

Math (reference):
    xn = x / ||x||                     # row-L2-normalized embeddings
    wf = xn @ W.T                      # [N, C] logits
    tgt = wf[i, y_i]
    num = S * (tgt - M)
    excl = sum_c exp(S*wf) - exp(S*tgt)
    L = num - log(exp(num) + excl);  loss = -mean(L)

Strategy: pure data-parallel over N (16384 rows -> 2048/core), no
collectives.  Each core computes the full [2048, 10000] logit block in
PSUM tiles (bf16 matmul, fp32 accumulate) and fuses exp + row-sum into
single ScalarEngine activation instructions:
    exp(S*r_i*raw_logit) via per-partition scale (S*r_i), accum_out=row-sum
The normalization 1/||x_i|| is folded into the activation scale, so x is
never normalized explicitly.  The target logit is computed exactly from a
host-side gather G = W[labels] via a DVE dot-product (x.G per row).
Per-row L values are DMA'd out; the host just concatenates and means.
"""

import numpy as np
import ml_dtypes

import concourse.mybir as mybir
import concourse.tile as tile
from concourse import bacc
from concourse.bass_utils import run_bass_kernel_spmd

N, D, C = 16384, 256, 10000
S, M = 30.0, 0.4
NCORES = 8
NS = N // NCORES      # 2048 rows per core
NT = NS // 128        # 16 n-tiles of 128 rows
KT = D // 128         # 2 k-slices
CG = [2048, 2048, 2048, 2048, 1808]   # class-dim groups (sum = C)
assert sum(CG) == C

_F32 = mybir.dt.float32
_BF16 = mybir.dt.bfloat16

N_WARMUP_MM = 24      # dummy matmuls to pull the PE HAM clock-gate to 8/8


def _build_nc(ns=NS, cg=tuple(CG), c=C):
    nt = ns // 128
    cg = list(cg)
    assert sum(cg) == c
    nc = bacc.Bacc("TRN2", target_bir_lowering=False)
    AF = mybir.ActivationFunctionType
    NT, C = nt, c  # noqa: N806 (shadow module constants for the body below)
    CG, NS = cg, ns  # noqa: N806

    xt_ext = nc.declare_dram_parameter("xt", [KT, 128, NS], _BF16, isOutput=False)
    wt_ext = nc.declare_dram_parameter("wt", [KT, 128, C], _BF16, isOutput=False)
    xf_ext = nc.declare_dram_parameter("xf", [NT, 128, D], _BF16, isOutput=False)
    g_ext = nc.declare_dram_parameter("g", [NT, 128, D], _BF16, isOutput=False)
    out_ext = nc.declare_dram_parameter("out", [128, NT], _F32, isOutput=True)

    with tile.TileContext(nc) as tc:
        with (
            tc.tile_pool(name="big", bufs=1) as big,
            tc.tile_pool(name="stat", bufs=1) as stat,
            tc.tile_pool(name="scr", bufs=1) as scr,
            tc.tile_pool(name="psum", bufs=2, space="PSUM") as psum,
        ):
            # ---- PE warmup: dummy matmuls on zeroed tiles so the HAM
            # clock-gate reaches 8/8 while DMAs land ----
            wu_a = scr.tile([128, 128], _BF16)
            wu_b = scr.tile([128, 512], _BF16)
            nc.vector.memset(wu_a, 0.0)
            nc.vector.memset(wu_b, 0.0)
            wu_p = psum.tile([128, 2048], _F32, tag="pt")
            for i in range(N_WARMUP_MM):
                nc.tensor.matmul(
                    wu_p[:, (i % 4) * 512 : (i % 4) * 512 + 512],
                    wu_a,
                    wu_b,
                    start=True,
                    stop=True,
                )

            # ---- input DMAs (phase-1 deps first, then weights) ----
            xf_sb = big.tile([128, NT, D], _BF16)
            g_sb = big.tile([128, NT, D], _BF16)
            wt_sb = big.tile([128, KT, C], _BF16)
            xt_sb = big.tile([128, KT, NS], _BF16)
            for t in range(NT):
                nc.sync.dma_start(out=xf_sb[:, t, :], in_=xf_ext[t])
                nc.sync.dma_start(out=g_sb[:, t, :], in_=g_ext[t])
            for k in range(KT):
                nc.sync.dma_start(out=xt_sb[:, k, :], in_=xt_ext[k])
            c0 = 0
            for w in CG:
                for k in range(KT):
                    nc.sync.dma_start(
                        out=wt_sb[:, k, c0 : c0 + w], in_=wt_ext[k, :, c0 : c0 + w]
                    )
                c0 += w

            # ---- phase 1: row stats (per-partition scalars, [128, NT]) ----
            ss = stat.tile([128, NT], _F32)     # sum of squares
            rawt = stat.tile([128, NT], _F32)   # x . W[y]  (unnormalized target)
            dot_scr = scr.tile([128, D], _F32)
            for t in range(NT):
                nc.vector.tensor_mul(dot_scr, xf_sb[:, t, :], xf_sb[:, t, :])
                nc.vector.reduce_sum(
                    ss[:, t : t + 1], dot_scr, axis=mybir.AxisListType.X
                )
                nc.vector.tensor_mul(dot_scr, xf_sb[:, t, :], g_sb[:, t, :])
                nc.vector.reduce_sum(
                    rawt[:, t : t + 1], dot_scr, axis=mybir.AxisListType.X
                )
            inv = stat.tile([128, NT], _F32)
            nc.vector.reciprocal(inv, ss)
            r = stat.tile([128, NT], _F32)
            nc.scalar.activation(r, inv, AF.Sqrt)          # r = 1/||x||
            sr = stat.tile([128, NT], _F32)
            nc.vector.tensor_scalar_mul(sr, r, S)          # S * r
            st = stat.tile([128, NT], _F32)
            nc.vector.tensor_mul(st, sr, rawt)             # S * tgt
            num = stat.tile([128, NT], _F32)
            nc.vector.tensor_scalar_add(num, st, -S * M)   # S * (tgt - M)
            expn = stat.tile([128, NT], _F32)
            nc.scalar.activation(expn, num, AF.Exp)
            expt = stat.tile([128, NT], _F32)
            nc.scalar.activation(expt, st, AF.Exp)

            # ---- phase 2: logits + fused exp/row-sum ----
            esum = stat.tile([128, NT * len(CG)], _F32)
            exp_scr = scr.tile([128, 2048], _F32)
            for gi, w in enumerate(CG):
                c0 = sum(CG[:gi])
                for t in range(NT):
                    pt = psum.tile([128, 2048], _F32, tag="pt")
                    for b0 in range(0, w, 512):
                        bw = min(512, w - b0)
                        for k in range(KT):
                            nc.tensor.matmul(
                                pt[:, b0 : b0 + bw],
                                xt_sb[:, k, t * 128 : (t + 1) * 128],
                                wt_sb[:, k, c0 + b0 : c0 + b0 + bw],
                                start=(k == 0),
                                stop=(k == KT - 1),
                            )
                    idx = t * len(CG) + gi
                    nc.scalar.activation(
                        exp_scr[:, :w],
                        pt[:, :w],
                        AF.Exp,
                        scale=sr[:, t : t + 1],
                        accum_out=esum[:, idx : idx + 1],
                    )

            # ---- phase 3: combine, log, write out ----
            esum_v = esum.rearrange("p (t g) -> p t g", g=len(CG))
            et = stat.tile([128, NT], _F32)
            for t in range(NT):
                nc.vector.reduce_sum(
                    et[:, t : t + 1], esum_v[:, t, :], axis=mybir.AxisListType.X
                )
            denom = stat.tile([128, NT], _F32)
            nc.vector.tensor_add(denom, et, expn)
            nc.vector.tensor_sub(denom, denom, expt)
            lg = stat.tile([128, NT], _F32)
            nc.scalar.activation(lg, denom, AF.Ln)
            L = stat.tile([128, NT], _F32)
            nc.vector.tensor_sub(L, num, lg)
            nc.sync.dma_start(out=out_ext[:], in_=L)

    nc.finalize()
    return nc


_NC_CACHE = None


def _get_nc():
    global _NC_CACHE
    if _NC_CACHE is None:
        _NC_CACHE = _build_nc()
    return _NC_CACHE


def make_in_maps(x, labels, W):
    x = np.asarray(x, dtype=np.float32)
    W = np.asarray(W, dtype=np.float32)
    labels = np.asarray(labels)
    wt = np.ascontiguousarray(W.T).astype(ml_dtypes.bfloat16).reshape(KT, 128, C)
    in_maps = []
    for i in range(NCORES):
        xs = x[i * NS : (i + 1) * NS]                       # [NS, D]
        ls = labels[i * NS : (i + 1) * NS]
        xt = np.ascontiguousarray(xs.T).astype(ml_dtypes.bfloat16).reshape(KT, 128, NS)
        xf = xs.astype(ml_dtypes.bfloat16).reshape(NT, 128, D)
        g = W[ls].astype(ml_dtypes.bfloat16).reshape(NT, 128, D)
        in_maps.append({"xt": xt, "wt": wt, "xf": xf, "g": g})
    return in_maps


def run_device(x, labels, W, **kwargs):
    nc = _get_nc()
    in_maps = make_in_maps(x, labels, W)
    res = run_bass_kernel_spmd(nc, in_maps, list(range(NCORES)), **kwargs)
    return res


def finish(res):
    parts = []
    for i in range(NCORES):
        o = res.results[i]["out"]            # [128, NT]; row = t*128 + p
        parts.append(np.asarray(o).T.reshape(-1))
    L = np.concatenate(parts)
    return np.asarray(-np.mean(L), dtype=np.float32)


def kernel(x, labels, W):
    return finish(run_device(x, labels, W))


# revision 8
# speedup vs baseline: 1.1326x; 1.0379x over previous
"""AdMSoftmaxLoss fused distributed kernel for 8 TRN2 NeuronCores.

Math (reference):
    xn = x / ||x||                     # row-L2-normalized embeddings
    wf = xn @ W.T                      # [N, C] logits
    tgt = wf[i, y_i]
    num = S * (tgt - M)
    excl = sum_c exp(S*wf) - exp(S*tgt)
    L = num - log(exp(num) + excl);  loss = -mean(L)

Strategy: pure data-parallel over N (16384 rows -> 2048/core), no
collectives.  Each core computes the full [2048, 10000] logit block in
PSUM tiles (bf16 matmul, fp32 accumulate) and fuses exp + row-sum into
single ScalarEngine activation instructions:
    exp(S*r_i*raw_logit) via per-partition scale (S*r_i), accum_out=row-sum
The normalization 1/||x_i|| is folded into the activation scale, so x is
never normalized explicitly.  The target logit is computed exactly from a
host-side gather G = W[labels] via a DVE dot-product (x.G per row).
Per-row L values are DMA'd out; the host just concatenates and means.
"""

import numpy as np
import ml_dtypes

import concourse.mybir as mybir
import concourse.tile as tile
from concourse import bacc
from concourse.bass_utils import run_bass_kernel_spmd

N, D, C = 16384, 256, 10000
S, M = 30.0, 0.4
NCORES = 8
NS = N // NCORES      # 2048 rows per core
NT = NS // 128        # 16 n-tiles of 128 rows
KT = D // 128         # 2 k-slices
CG = [2048, 2048, 2048, 2048, 1808]   # class-dim groups (sum = C)
assert sum(CG) == C

_F32 = mybir.dt.float32
_BF16 = mybir.dt.bfloat16

N_WARMUP_MM = 24      # dummy matmuls to pull the PE HAM clock-gate to 8/8


def _build_nc(ns=NS, cg=tuple(CG), c=C):
    nt = ns // 128
    cg = list(cg)
    assert sum(cg) == c
    nc = bacc.Bacc("TRN2", target_bir_lowering=False)
    AF = mybir.ActivationFunctionType
    NT, C = nt, c  # noqa: N806 (shadow module constants for the body below)
    CG, NS = cg, ns  # noqa: N806

    xt_ext = nc.declare_dram_parameter("xt", [KT, 128, NS], _BF16, isOutput=False)
    wt_ext = nc.declare_dram_parameter("wt", [KT, 128, C], _BF16, isOutput=False)
    xf_ext = nc.declare_dram_parameter("xf", [NT, 128, D], _BF16, isOutput=False)
    g_ext = nc.declare_dram_parameter("g", [NT, 128, D], _BF16, isOutput=False)
    out_ext = nc.declare_dram_parameter("out", [128, NT], _F32, isOutput=True)

    mult = mybir.AluOpType.mult
    sub = mybir.AluOpType.subtract
    asr = mybir.AluOpType.arith_shift_right
    addop = mybir.AluOpType.add
    _I32 = mybir.dt.int32

    with tile.TileContext(nc) as tc:
        with (
            tc.tile_pool(name="big", bufs=1) as big,
            tc.tile_pool(name="stat", bufs=1) as stat,
            tc.tile_pool(name="scr", bufs=1) as scr,
            tc.tile_pool(name="psum", bufs=2, space="PSUM") as psum,
        ):
            # ---- prologue: warm the exp ACT table + the PE HAM clock-gate
            # while the first DMAs land ----
            wu_a = scr.tile([128, 128], _BF16)
            wu_b = scr.tile([128, 512], _BF16)
            wu_e = scr.tile([128, 1], _F32)
            nc.vector.memset(wu_a, 0.0)
            nc.vector.memset(wu_b, 0.0)
            nc.vector.memset(wu_e, 1.0)
            nc.scalar.activation(wu_e, wu_e, AF.Ln)   # pull ln/exp table load early
            nc.scalar.activation(wu_e, wu_e, AF.Exp)
            wu_p = psum.tile([128, 2048], _F32, tag="pt")
            for i in range(N_WARMUP_MM):
                nc.tensor.matmul(
                    wu_p[:, (i % 4) * 512 : (i % 4) * 512 + 512],
                    wu_a,
                    wu_b,
                    start=True,
                    stop=True,
                )

            # ---- input DMAs, ordered by when they gate compute:
            # xf (row stats) -> xt (lhsT) -> wt chunk0 -> rest of wt -> g ----
            xf_sb = big.tile([128, NT, D], _BF16)
            g_sb = big.tile([128, NT, D], _BF16)
            wt_sb = big.tile([128, KT, C], _BF16)
            xt_sb = big.tile([128, KT, NS], _BF16)
            for t in range(NT):
                nc.sync.dma_start(out=xf_sb[:, t, :], in_=xf_ext[t])
            for k in range(KT):
                nc.sync.dma_start(out=xt_sb[:, k, :], in_=xt_ext[k])
            c0 = 0
            for w in CG:
                for k in range(KT):
                    nc.sync.dma_start(
                        out=wt_sb[:, k, c0 : c0 + w], in_=wt_ext[k, :, c0 : c0 + w]
                    )
                c0 += w
            for t in range(NT):
                nc.sync.dma_start(out=g_sb[:, t, :], in_=g_ext[t])

            # ---- phase 1: sum-of-squares per row, then S/||x|| via a
            # DVE-only Newton rsqrt (no ACT table switch), in two halves so
            # the exp stream can start after the first half ----
            ss = stat.tile([128, NT], _F32)
            sr = stat.tile([128, NT], _F32)      # S / ||x||
            dot_scr = scr.tile([128, D], _F32)
            yv = stat.tile([128, NT], _F32)
            t1 = stat.tile([128, NT], _F32)
            t2 = stat.tile([128, NT], _F32)

            def _rsqrt_half(h):
                lo, hi = h * (NT // 2), (h + 1) * (NT // 2)
                ssh = ss[:, lo:hi]
                yh, t1h, t2h = yv[:, lo:hi], t1[:, lo:hi], t2[:, lo:hi]
                # quake seed: y0 = bitcast(0x5f3759df - (bitcast(ss) >> 1))
                nc.vector.tensor_scalar(
                    t1h.bitcast(_I32), ssh.bitcast(_I32), 1, None, asr
                )
                nc.vector.tensor_scalar(
                    yh.bitcast(_I32), t1h.bitcast(_I32), 0x5F3759DF, -1, sub, mult
                )
                for _ in range(2):  # Newton: y *= 1.5 - 0.5*ss*y^2
                    nc.vector.tensor_mul(t1h, yh, yh)
                    nc.vector.tensor_mul(t2h, t1h, ssh)
                    nc.vector.tensor_scalar(t1h, t2h, -0.5, 1.5, mult, addop)
                    nc.vector.tensor_mul(yh, yh, t1h)
                nc.vector.tensor_scalar_mul(sr[:, lo:hi], yh, S)

            for h in range(2):
                for t in range(h * (NT // 2), (h + 1) * (NT // 2)):
                    nc.vector.tensor_mul(dot_scr, xf_sb[:, t, :], xf_sb[:, t, :])
                    nc.vector.reduce_sum(
                        ss[:, t : t + 1], dot_scr, axis=mybir.AxisListType.X
                    )
                _rsqrt_half(h)

            # ---- phase 2: logits + fused exp/row-sum ----
            esum = stat.tile([128, NT * len(CG)], _F32)
            exp_scr = scr.tile([128, 2048], _F32)

            def _group_pass(gi, w):
                c0 = sum(CG[:gi])
                for t in range(NT):
                    pt = psum.tile([128, 2048], _F32, tag="pt")
                    for b0 in range(0, w, 512):
                        bw = min(512, w - b0)
                        for k in range(KT):
                            nc.tensor.matmul(
                                pt[:, b0 : b0 + bw],
                                xt_sb[:, k, t * 128 : (t + 1) * 128],
                                wt_sb[:, k, c0 + b0 : c0 + b0 + bw],
                                start=(k == 0),
                                stop=(k == KT - 1),
                            )
                    idx = t * len(CG) + gi
                    nc.scalar.activation(
                        exp_scr[:, :w],
                        pt[:, :w],
                        AF.Exp,
                        scale=sr[:, t : t + 1],
                        accum_out=esum[:, idx : idx + 1],
                    )

            _group_pass(0, CG[0])

            # target-logit path (only needed for phase 3; DVE runs it while
            # the PE/ACT stream continues)
            rawt = stat.tile([128, NT], _F32)
            for t in range(NT):
                nc.vector.tensor_mul(dot_scr, xf_sb[:, t, :], g_sb[:, t, :])
                nc.vector.reduce_sum(
                    rawt[:, t : t + 1], dot_scr, axis=mybir.AxisListType.X
                )
            st = stat.tile([128, NT], _F32)
            nc.vector.tensor_mul(st, sr, rawt)             # S * tgt
            num = stat.tile([128, NT], _F32)
            nc.vector.tensor_scalar_add(num, st, -S * M)   # S * (tgt - M)
            expn = stat.tile([128, NT], _F32)
            nc.scalar.activation(expn, num, AF.Exp)
            expt = stat.tile([128, NT], _F32)
            nc.scalar.activation(expt, st, AF.Exp)

            for gi, w in enumerate(CG):
                if gi > 0:
                    _group_pass(gi, w)

            # ---- phase 3: combine, log, write out ----
            esum_v = esum.rearrange("p (t g) -> p t g", g=len(CG))
            et = stat.tile([128, NT], _F32)
            for t in range(NT):
                nc.vector.reduce_sum(
                    et[:, t : t + 1], esum_v[:, t, :], axis=mybir.AxisListType.X
                )
            denom = stat.tile([128, NT], _F32)
            nc.vector.tensor_add(denom, et, expn)
            nc.vector.tensor_sub(denom, denom, expt)
            lg = stat.tile([128, NT], _F32)
            nc.scalar.activation(lg, denom, AF.Ln)
            L = stat.tile([128, NT], _F32)
            nc.vector.tensor_sub(L, num, lg)
            nc.sync.dma_start(out=out_ext[:], in_=L)

    nc.finalize()
    return nc


_NC_CACHE = None


def _get_nc():
    global _NC_CACHE
    if _NC_CACHE is None:
        _NC_CACHE = _build_nc()
    return _NC_CACHE


def make_in_maps(x, labels, W):
    x = np.asarray(x, dtype=np.float32)
    W = np.asarray(W, dtype=np.float32)
    labels = np.asarray(labels)
    wt = np.ascontiguousarray(W.T).astype(ml_dtypes.bfloat16).reshape(KT, 128, C)
    in_maps = []
    for i in range(NCORES):
        xs = x[i * NS : (i + 1) * NS]                       # [NS, D]
        ls = labels[i * NS : (i + 1) * NS]
        xt = np.ascontiguousarray(xs.T).astype(ml_dtypes.bfloat16).reshape(KT, 128, NS)
        xf = xs.astype(ml_dtypes.bfloat16).reshape(NT, 128, D)
        g = W[ls].astype(ml_dtypes.bfloat16).reshape(NT, 128, D)
        in_maps.append({"xt": xt, "wt": wt, "xf": xf, "g": g})
    return in_maps


def run_device(x, labels, W, **kwargs):
    nc = _get_nc()
    in_maps = make_in_maps(x, labels, W)
    res = run_bass_kernel_spmd(nc, in_maps, list(range(NCORES)), **kwargs)
    return res


def finish(res):
    parts = []
    for i in range(NCORES):
        o = res.results[i]["out"]            # [128, NT]; row = t*128 + p
        parts.append(np.asarray(o).T.reshape(-1))
    L = np.concatenate(parts)
    return np.asarray(-np.mean(L), dtype=np.float32)


def kernel(x, labels, W):
    return finish(run_device(x, labels, W))


# revision 10
# speedup vs baseline: 1.2369x; 1.0921x over previous
"""AdMSoftmaxLoss fused distributed kernel for 8 TRN2 NeuronCores.

Math (reference):
    xn = x / ||x||                     # row-L2-normalized embeddings
    wf = xn @ W.T                      # [N, C] logits
    tgt = wf[i, y_i]
    num = S * (tgt - M)
    excl = sum_c exp(S*wf) - exp(S*tgt)
    L = num - log(exp(num) + excl);  loss = -mean(L)

Strategy: pure data-parallel over N (16384 rows -> 2048/core), no
collectives.  Each core computes its full [2048, 10000] logit block in
PSUM (bf16 matmul, fp32 accumulate) and applies exp with the ScalarEngine
activation, folding S/||x_i|| in as the per-partition activation scale.
Row sums of exp are split between the ACT accumulator (accum_out) and
VectorEngine reductions of the bf16 exp output, so ACT and DVE share the
reduction load.  The target logit is computed from a host-side gather
G = W[labels] via DVE dot products; per-row L values are DMA'd out and
the host just concatenates and means.

All device inputs are pre-shuffled on the host to partition-major
layouts so every DMA is a large contiguous transfer.
"""

import numpy as np
import ml_dtypes

import concourse.mybir as mybir
import concourse.tile as tile
from concourse import bacc
from concourse.bass_utils import run_bass_kernel_spmd

N, D, C = 16384, 256, 10000
S, M = 30.0, 0.4
NCORES = 8
NS = N // NCORES      # 2048 rows per core
NT = NS // 128        # 16 n-tiles of 128 rows
KT = D // 128         # 2 k-slices
CG = [2048, 2048, 2048, 2048, 1808]   # class-dim groups (sum = C)
assert sum(CG) == C

_F32 = mybir.dt.float32
_BF16 = mybir.dt.bfloat16
_I32 = mybir.dt.int32

N_WARMUP_MM = 10      # dummy matmuls to pull the PE HAM clock-gate to 8/8
ACT_ACCUM_EVERY = 4   # 1 of every this many groups reduced via ACT accum_out


def _build_nc(ns=NS, cg=tuple(CG), c=C):
    nt = ns // 128
    cg = list(cg)
    assert sum(cg) == c
    nc = bacc.Bacc("TRN2", target_bir_lowering=False)
    AF = mybir.ActivationFunctionType
    NT, C = nt, c  # noqa: N806 (shadow module constants for the body below)
    CG, NS = cg, ns  # noqa: N806
    NG = len(CG)  # noqa: N806
    NH = NT // 2  # noqa: N806
    mult = mybir.AluOpType.mult
    sub = mybir.AluOpType.subtract
    asr = mybir.AluOpType.arith_shift_right
    addop = mybir.AluOpType.add

    xt_ext = nc.declare_dram_parameter("xt", [128, KT, NS], _BF16, isOutput=False)
    wt_ext = nc.declare_dram_parameter("wt", [128, KT, C], _BF16, isOutput=False)
    xf_ext = nc.declare_dram_parameter("xf", [128, NT, D], _BF16, isOutput=False)
    g_ext = nc.declare_dram_parameter("g", [128, NT, D], _BF16, isOutput=False)
    out_ext = nc.declare_dram_parameter("out", [128, NT], _F32, isOutput=True)

    with tile.TileContext(nc) as tc:
        with (
            tc.tile_pool(name="big", bufs=1) as big,
            tc.tile_pool(name="stat", bufs=1) as stat,
            tc.tile_pool(name="scr", bufs=1) as scr,
            tc.tile_pool(name="expb", bufs=4) as expb,
            tc.tile_pool(name="psum", bufs=2, space="PSUM") as psum,
        ):
            # ---- prologue: warm the exp ACT table + the PE HAM clock-gate
            # while the first DMAs land ----
            wu_a = scr.tile([128, 128], _BF16)
            wu_b = scr.tile([128, 512], _BF16)
            wu_e = scr.tile([128, 1], _F32)
            nc.vector.memset(wu_a, 0.0)
            nc.vector.memset(wu_b, 0.0)
            nc.vector.memset(wu_e, 0.0)
            nc.scalar.activation(wu_e, wu_e, AF.Exp)  # pull exp table load early
            wu_p = psum.tile([128, 2048], _F32, tag="pt")
            for i in range(N_WARMUP_MM):
                nc.tensor.matmul(
                    wu_p[:, (i % 4) * 512 : (i % 4) * 512 + 512],
                    wu_a,
                    wu_b,
                    start=True,
                    stop=True,
                )

            # ---- input DMAs, ordered by when they gate compute ----
            xf_sb = big.tile([128, NT, D], _BF16)
            g_sb = big.tile([128, NT, D], _BF16)
            wt_sb = big.tile([128, KT, C], _BF16)
            xt_sb = big.tile([128, KT, NS], _BF16)

            def _wt_chunk(gi):
                c0 = sum(CG[:gi])
                w = CG[gi]
                for k in range(KT):
                    nc.sync.dma_start(
                        out=wt_sb[:, k, c0 : c0 + w], in_=wt_ext[:, k, c0 : c0 + w]
                    )

            # critical chain: xf half 0 -> xt t0 cols -> wt chunk 0
            nc.sync.dma_start(out=xf_sb[:, :NH, :], in_=xf_ext[:, :NH, :])
            for k in range(KT):
                nc.sync.dma_start(out=xt_sb[:, k, :128], in_=xt_ext[:, k, :128])
            _wt_chunk(0)
            nc.sync.dma_start(out=xf_sb[:, NH:, :], in_=xf_ext[:, NH:, :])
            for k in range(KT):
                nc.sync.dma_start(out=xt_sb[:, k, 128:], in_=xt_ext[:, k, 128:])
            nc.sync.dma_start(out=g_sb[:, :NH, :], in_=g_ext[:, :NH, :])
            nc.sync.dma_start(out=g_sb[:, NH:, :], in_=g_ext[:, NH:, :])
            for gi in range(1, NG):
                _wt_chunk(gi)

            # ---- phase 1: ||x||^2 per row, then S/||x|| via a DVE-only
            # Newton rsqrt (no ACT table switch), in halves so the exp
            # stream can start after the first half ----
            ss = stat.tile([128, NT], _F32)
            sr = stat.tile([128, NT], _F32)      # S / ||x||
            sq_scr = scr.tile([128, NH, D], _F32)
            yv = stat.tile([128, NT], _F32)
            t1 = stat.tile([128, NT], _F32)
            t2 = stat.tile([128, NT], _F32)

            def _rsqrt(lo, hi, out_scale):
                ssh = ss[:, lo:hi]
                yh, t1h, t2h = yv[:, lo:hi], t1[:, lo:hi], t2[:, lo:hi]
                # quake seed: y0 = bitcast(0x5f3759df - (bitcast(ss) >> 1))
                nc.vector.tensor_scalar(
                    t1h.bitcast(_I32), ssh.bitcast(_I32), 1, None, asr
                )
                nc.vector.tensor_scalar(
                    yh.bitcast(_I32), t1h.bitcast(_I32), 0x5F3759DF, -1, sub, mult
                )
                for _ in range(2):  # Newton: y *= 1.5 - 0.5*ss*y^2
                    nc.vector.tensor_mul(t1h, yh, yh)
                    nc.vector.tensor_mul(t2h, t1h, ssh)
                    nc.vector.tensor_scalar(t1h, t2h, -0.5, 1.5, mult, addop)
                    nc.vector.tensor_mul(yh, yh, t1h)
                nc.vector.tensor_scalar_mul(out_scale[:, lo:hi], yh, S)

            for h in range(2):
                lo, hi = h * NH, (h + 1) * NH
                nc.vector.tensor_mul(sq_scr, xf_sb[:, lo:hi, :], xf_sb[:, lo:hi, :])
                nc.vector.reduce_sum(
                    ss[:, lo:hi], sq_scr, axis=mybir.AxisListType.X
                )
                _rsqrt(lo, hi, sr)

            # ---- phase 2: logits + exp; row-sums split ACT-accum / DVE ----
            esum = stat.tile([128, NT * NG], _F32)

            def _group_pass(gi, w):
                c0 = sum(CG[:gi])
                for t in range(NT):
                    it = gi * NT + t
                    pt = psum.tile([128, 2048], _F32, tag="pt")
                    for b0 in range(0, w, 512):
                        bw = min(512, w - b0)
                        for k in range(KT):
                            nc.tensor.matmul(
                                pt[:, b0 : b0 + bw],
                                xt_sb[:, k, t * 128 : (t + 1) * 128],
                                wt_sb[:, k, c0 + b0 : c0 + b0 + bw],
                                start=(k == 0),
                                stop=(k == KT - 1),
                            )
                    idx = t * NG + gi
                    eo = expb.tile([128, 2048], _BF16, tag="ex")
                    if it % ACT_ACCUM_EVERY == 0:
                        nc.scalar.activation(
                            eo[:, :w],
                            pt[:, :w],
                            AF.Exp,
                            scale=sr[:, t : t + 1],
                            accum_out=esum[:, idx : idx + 1],
                        )
                    else:
                        nc.scalar.activation(
                            eo[:, :w], pt[:, :w], AF.Exp, scale=sr[:, t : t + 1]
                        )
                        nc.vector.reduce_sum(
                            esum[:, idx : idx + 1],
                            eo[:, :w],
                            axis=mybir.AxisListType.X,
                        )

            _group_pass(0, CG[0])

            # target-logit path (feeds only phase 3; runs on DVE while the
            # PE/ACT stream continues)
            rawt = stat.tile([128, NT], _F32)
            for h in range(2):
                lo, hi = h * NH, (h + 1) * NH
                nc.vector.tensor_mul(sq_scr, xf_sb[:, lo:hi, :], g_sb[:, lo:hi, :])
                nc.vector.reduce_sum(
                    rawt[:, lo:hi], sq_scr, axis=mybir.AxisListType.X
                )
            st = stat.tile([128, NT], _F32)
            nc.vector.tensor_mul(st, sr, rawt)             # S * tgt
            num = stat.tile([128, NT], _F32)
            nc.vector.tensor_scalar_add(num, st, -S * M)   # S * (tgt - M)
            expn = stat.tile([128, NT], _F32)
            nc.scalar.activation(expn, num, AF.Exp)
            expt = stat.tile([128, NT], _F32)
            nc.scalar.activation(expt, st, AF.Exp)

            for gi in range(1, NG):
                _group_pass(gi, CG[gi])

            # ---- phase 3: combine, log, write out ----
            esum_v = esum.rearrange("p (t g) -> p t g", g=NG)
            et = stat.tile([128, NT], _F32)
            nc.vector.reduce_sum(et, esum_v, axis=mybir.AxisListType.X)
            denom = stat.tile([128, NT], _F32)
            nc.vector.tensor_add(denom, et, expn)
            nc.vector.tensor_sub(denom, denom, expt)
            lg = stat.tile([128, NT], _F32)
            nc.scalar.activation(lg, denom, AF.Ln)
            L = stat.tile([128, NT], _F32)
            nc.vector.tensor_sub(L, num, lg)
            nc.sync.dma_start(out=out_ext[:], in_=L)

    nc.finalize()
    return nc


_NC_CACHE = None


def _get_nc():
    global _NC_CACHE
    if _NC_CACHE is None:
        _NC_CACHE = _build_nc()
    return _NC_CACHE


def _shuffle_pm(a, nt):
    """[nt*128, d] row-major -> [128, nt, d] partition-major."""
    d = a.shape[-1]
    return np.ascontiguousarray(a.reshape(nt, 128, d).transpose(1, 0, 2))


def prep_core(xs, ls, W, wt=None):
    """Build one core's input map from its row block. Layouts partition-major."""
    nt = xs.shape[0] // 128
    c = W.shape[0]
    if wt is None:
        wt = _shuffle_pm(np.ascontiguousarray(W.T), KT).astype(ml_dtypes.bfloat16)
    xt = _shuffle_pm(np.ascontiguousarray(xs.T), KT).astype(ml_dtypes.bfloat16)
    xf = _shuffle_pm(xs, nt).astype(ml_dtypes.bfloat16)
    g = _shuffle_pm(W[ls], nt).astype(ml_dtypes.bfloat16)
    return {"xt": xt, "wt": wt, "xf": xf, "g": g}


def make_in_maps(x, labels, W):
    x = np.asarray(x, dtype=np.float32)
    W = np.asarray(W, dtype=np.float32)
    labels = np.asarray(labels)
    wt = _shuffle_pm(np.ascontiguousarray(W.T), KT).astype(ml_dtypes.bfloat16)
    return [
        prep_core(
            x[i * NS : (i + 1) * NS], labels[i * NS : (i + 1) * NS], W, wt
        )
        for i in range(NCORES)
    ]


def run_device(x, labels, W, **kwargs):
    nc = _get_nc()
    in_maps = make_in_maps(x, labels, W)
    res = run_bass_kernel_spmd(nc, in_maps, list(range(NCORES)), **kwargs)
    return res


def finish(res):
    parts = []
    for i in range(NCORES):
        o = res.results[i]["out"]            # [128, NT]; row = t*128 + p
        parts.append(np.asarray(o).T.reshape(-1))
    L = np.concatenate(parts)
    return np.asarray(-np.mean(L), dtype=np.float32)


def kernel(x, labels, W):
    return finish(run_device(x, labels, W))
